# revision 9
# baseline (speedup 1.0000x reference)
"""Trainium2 Bass kernel for nn_ChannelMerger.

Computation (per batch b):
    emb   = fourier_emb(positions[b])            # [C, D]   D=288
    scores= emb @ heads.T                        # [C, O]   O=270 (kept transposed)
    w     = softmax(scores + mask_offset, axis=C)
    out[b]= (w.T @ meg[b])                       # [O, T]

Sharding: data-parallel over batch B=32 across 8 cores (4 batches/core).
heads + fourier constants replicated.  All compute on-device; softmax
normalization is folded into the PSUM->SBUF evacuation of the final
matmul (scale by 1/sum_exp per output row).

Fourier embedding on device:
    loc'[d, c] = x_c*px[d] + y_c*py[d] + (margin*(px+py)[d] + 2*pi*phase[d])
  computed as a K=3 matmul with a host-precomputed constant matrix p3t
  ([3, 288]: rows px, py, const) against [x; y; ones] ([3, C]).
  phase = 0.25 turns for the cos half (d<144), 0 for the sin half.
  Then t = loc'/(2*pi); r = round(t) via the +-1.5*2^23 magic trick;
  emb = Sin(2*pi*(t - r)) with the argument guaranteed in [-pi, pi].
"""

import math

import numpy as np

import concourse.bacc as bacc
import concourse.bass as bass
import concourse.mybir as mybir
from concourse.bass_utils import run_bass_kernel_spmd
from concourse.tile import TileContext

# Problem shape (hardcoded per contract)
B, C, T = 32, 273, 4096
O, D = 270, 288
NF = 12            # fourier freqs per axis (sqrt(D/2))
MARGIN = 0.1
NCORES = 8
BPC = B // NCORES  # batches per core

TT = 2048          # T tile (columns of the big matmul kept in SBUF at once)
NT = T // TT
MM_N = 512         # moving free dim per matmul / one PSUM bank of fp32

C_CHUNKS = [(0, 128), (128, 128), (256, C - 256)]    # K chunks of the big matmul
O_CHUNKS = [(0, 128), (128, 128), (256, O - 256)]    # M chunks of the big matmul
D_CHUNKS = [(0, 128), (128, 128), (256, D - 256)]    # K chunks of the scores matmul

MAGIC = 1.5 * 2.0**23       # fp32 round-to-nearest-integer magic constant
TWO_PI = 2.0 * math.pi
NEG_BIG = -1.0e30           # stands in for -inf on masked channels

F32 = mybir.dt.float32
F32R = mybir.dt.float32r

USE_FP32R = True            # big matmul in fp32r (4x faster than fp32 on PE)

_CACHE = {}
LAST_RESULTS = None         # BassKernelResults of the most recent run (for test.py)


def _fourier_consts():
    """p3t [3, D]: rows px, py, and the per-d additive constant."""
    p = (2.0 * math.pi / (1.0 + 2.0 * MARGIN)) * np.arange(NF, dtype=np.float64)
    dd = np.arange(D) % (NF * NF)
    fx, fy = dd // NF, dd % NF
    px, py = p[fx], p[fy]
    phase = np.where(np.arange(D) < NF * NF, 0.25, 0.0)  # cos half first
    const = MARGIN * (px + py) + TWO_PI * phase
    return np.stack([px, py, const]).astype(np.float32)


def _build_program():
    nc = bacc.Bacc(trn_type="TRN2", target_bir_lowering=False, debug=False)

    meg = nc.dram_tensor("meg", [BPC, C, T], F32, kind="ExternalInput").ap()
    posa = nc.dram_tensor("posa", [BPC, 3, C], F32, kind="ExternalInput").ap()
    maskf = nc.dram_tensor("maskf", [BPC, C], F32, kind="ExternalInput").ap()
    headsT = nc.dram_tensor("headsT", [D, O], F32, kind="ExternalInput").ap()
    p3t = nc.dram_tensor("p3t", [3, D], F32, kind="ExternalInput").ap()
    out = nc.dram_tensor("out", [BPC, O, T], F32, kind="ExternalOutput").ap()

    with TileContext(nc) as tc:
        with (
            tc.tile_pool(name="singles", bufs=1) as singles,
            tc.tile_pool(name="w", bufs=2) as wp,
            tc.tile_pool(name="megp", bufs=2) as megp,
            tc.tile_pool(name="outp", bufs=2) as outp,
            tc.tile_pool(name="psmall", bufs=2, space="PSUM") as psmall,
            tc.tile_pool(name="pssum", bufs=1, space="PSUM") as pssum,
            tc.tile_pool(name="psbig", bufs=5, space="PSUM") as psbig,
        ):
            BIG_DT = F32R if USE_FP32R else F32
            # ---- replicated constants ----
            p3t_sb = singles.tile([3, D], F32, name="p3t_sb")
            nc.sync.dma_start(out=p3t_sb, in_=p3t)
            ones_sb = singles.tile([128, 1], F32, name="ones_sb")
            nc.vector.memset(ones_sb, 1.0)
            headsT_sb = []
            for k, (d0, dn) in enumerate(D_CHUNKS):
                h = singles.tile([dn, O], F32, name=f"headsT_sb{k}")
                nc.sync.dma_start(out=h, in_=headsT[d0 : d0 + dn, :])
                headsT_sb.append(h)

            for b in range(BPC):
                # ---- per-batch softmax weights ----
                posT = wp.tile([3, C], F32, name=f"posT_b{b}", tag="posT")
                nc.sync.dma_start(out=posT, in_=posa[b])

                embT = []
                for k, (d0, dn) in enumerate(D_CHUNKS):
                    locp = psmall.tile([dn, C], F32, name=f"locp_b{b}k{k}", tag="sc")
                    nc.tensor.matmul(
                        locp, p3t_sb[:, d0 : d0 + dn], posT, start=True, stop=True
                    )
                    tt_ = wp.tile([dn, C], F32, name=f"tt_b{b}k{k}", tag="tt", bufs=3)
                    nc.vector.tensor_scalar_mul(tt_, locp, 1.0 / TWO_PI)
                    rr_ = wp.tile([dn, C], F32, name=f"rr_b{b}k{k}", tag="rr", bufs=3)
                    nc.vector.tensor_scalar(
                        rr_,
                        tt_,
                        MAGIC,
                        MAGIC,
                        op0=mybir.AluOpType.add,
                        op1=mybir.AluOpType.subtract,
                    )
                    dd_ = wp.tile([dn, C], F32, name=f"dd_b{b}k{k}", tag="dd", bufs=3)
                    nc.vector.tensor_sub(dd_, tt_, rr_)
                    e = wp.tile([dn, C], F32, name=f"embT_b{b}k{k}", tag=f"embT{k}")
                    nc.scalar.activation(
                        e, dd_, mybir.ActivationFunctionType.Sin, scale=TWO_PI
                    )
                    embT.append(e)

                expT = []
                for j, (c0, cn) in enumerate(C_CHUNKS):
                    offs = wp.tile([cn, 1], F32, name=f"offs_b{b}j{j}", tag=f"offs{j}")
                    nc.sync.dma_start(
                        out=offs, in_=maskf[b, c0 : c0 + cn].unsqueeze(-1)
                    )
                    nc.vector.tensor_scalar_mul(offs, offs, NEG_BIG)

                    sc = psmall.tile([cn, O], F32, name=f"sc_b{b}j{j}", tag="sc")
                    for k, (d0, dn) in enumerate(D_CHUNKS):
                        nc.tensor.matmul(
                            sc,
                            embT[k][:, c0 : c0 + cn],
                            headsT_sb[k],
                            start=(k == 0),
                            stop=(k == len(D_CHUNKS) - 1),
                        )
                    ex = wp.tile([cn, O], BIG_DT, name=f"expT_b{b}j{j}", tag=f"expT{j}")
                    nc.scalar.activation(
                        ex, sc, mybir.ActivationFunctionType.Exp, bias=offs
                    )
                    expT.append(ex)

                # sum over channels of exp -> [O] (via ones matmul), then 1/x
                sume = pssum.tile([128, len(O_CHUNKS)], F32, name=f"sume_b{b}", tag="sume")
                for oc, (o0, on) in enumerate(O_CHUNKS):
                    for j, (c0, cn) in enumerate(C_CHUNKS):
                        nc.tensor.matmul(
                            sume[0:on, oc : oc + 1],
                            expT[j][:, o0 : o0 + on].bitcast(F32),
                            ones_sb[0:cn, :],
                            start=(j == 0),
                            stop=(j == len(C_CHUNKS) - 1),
                        )
                inv = []
                for oc, (o0, on) in enumerate(O_CHUNKS):
                    iv = wp.tile([on, 1], F32, name=f"inv_b{b}o{oc}", tag=f"inv{oc}")
                    nc.vector.reciprocal(iv, sume[0:on, oc : oc + 1])
                    inv.append(iv)

                # ---- big matmul over T tiles ----
                for th in range(NT):
                    t0 = th * TT
                    megs = []
                    for j, (c0, cn) in enumerate(C_CHUNKS):
                        mg = megp.tile(
                            [cn, TT], BIG_DT, name=f"meg_b{b}t{th}j{j}", tag=f"meg{j}"
                        )
                        src = meg[b, c0 : c0 + cn, t0 : t0 + TT]
                        if USE_FP32R:
                            src = src.bitcast(F32R)
                        nc.sync.dma_start(out=mg, in_=src)
                        megs.append(mg)
                    for oc, (o0, on) in enumerate(O_CHUNKS):
                        ob = outp.tile(
                            [on, TT], F32, name=f"out_b{b}t{th}o{oc}", tag=f"out{oc}"
                        )
                        pbs = []
                        for nt in range(TT // MM_N):
                            pb = psbig.tile(
                                [on, MM_N], F32, name=f"pb_b{b}t{th}o{oc}n{nt}", tag="pb"
                            )
                            pbs.append(pb)
                        for j in range(len(C_CHUNKS)):
                            lhsT = expT[j][:, o0 : o0 + on]
                            for nt in range(TT // MM_N):
                                rhs = megs[j][:, nt * MM_N : (nt + 1) * MM_N]
                                nc.tensor.matmul(
                                    pbs[nt],
                                    lhsT,
                                    rhs,
                                    start=(j == 0),
                                    stop=(j == len(C_CHUNKS) - 1),
                                )
                        for nt in range(TT // MM_N):
                            dst = ob[:, nt * MM_N : (nt + 1) * MM_N]
                            if nt % 2 == 0:
                                nc.vector.tensor_scalar_mul(dst, pbs[nt], inv[oc])
                            else:
                                nc.scalar.activation(
                                    dst,
                                    pbs[nt],
                                    mybir.ActivationFunctionType.Copy,
                                    scale=inv[oc],
                                )
                        nc.sync.dma_start(
                            out=out[b, o0 : o0 + on, t0 : t0 + TT], in_=ob
                        )
    nc.compile()
    return nc


def _get_program():
    if "nc" not in _CACHE:
        _CACHE["nc"] = _build_program()
    return _CACHE["nc"]


def kernel(meg, positions, heads, invalid_mask, trace=False):
    global LAST_RESULTS
    meg = np.ascontiguousarray(meg, dtype=np.float32)
    positions = np.asarray(positions, dtype=np.float32)
    heads = np.asarray(heads, dtype=np.float32)

    headsT = np.ascontiguousarray(heads.T)                       # [D, O]
    p3t = _fourier_consts()                                      # [3, D]
    maskf = np.ascontiguousarray(invalid_mask.astype(np.float32))  # [B, C]
    # [B, 3, C]: rows x, y, ones
    posa = np.ascontiguousarray(
        np.stack(
            [positions[:, :, 0], positions[:, :, 1], np.ones((B, C), np.float32)],
            axis=1,
        ).astype(np.float32)
    )

    nc = _get_program()
    in_maps = []
    for c in range(NCORES):
        s = slice(c * BPC, (c + 1) * BPC)
        in_maps.append(
            {
                "meg": np.ascontiguousarray(meg[s]),
                "posa": np.ascontiguousarray(posa[s]),
                "maskf": np.ascontiguousarray(maskf[s]),
                "headsT": headsT,
                "p3t": p3t,
            }
        )

    res = run_bass_kernel_spmd(nc, in_maps, core_ids=list(range(NCORES)), trace=trace)
    LAST_RESULTS = res
    return np.concatenate([r["out"] for r in res.results], axis=0)
